# revision 1
# baseline (speedup 1.0000x reference)
"""Multi-head attention (B=2, S=2048, D=1024, H=16, dk=dv=64) on 8 TRN2 NeuronCores.

Sharding: core c -> (batch b = c//4, head-group g = c%4, 4 heads each).
Each core computes q/k/v projections for its 4 heads (weight-column shard),
attention over its batch, and a partial output projection over its 256
channels (weight-row shard of Wo).  The host sums the 4 partial outputs per
batch at unshard time (the "all-reduce after the output projection").

Host-side shard prep:
  * X slices are transposed to [D, S] so the contraction dim (D) lands on
    SBUF partitions for the projection matmuls.
  * The key-padding mask is applied by COMPACTION: masked keys are removed
    (gathered) from K/V before they ever reach the device.  This is exactly
    the reference semantics (masked keys get zero attention weight) and
    roughly halves the k-side work.
  * The softmax 1/sqrt(dk) scale is folded into Wq/bq.
  * All matmul operands are pre-rounded to fp32r (fp32 with 11-bit mantissa,
    low 12 bits zero) so the tensor engine runs fp32r matmuls at full rate.

Device softmax: scores are small (|s| ~ 10) so exp needs no max-subtraction.
The denominator comes for free as a 65th "ones" column appended to V; the
normalization divides the unnormalized context rows by that accumulated sum.
The output bias bo is added on the host during the partial-sum gather.
"""
import numpy as np

B, S, D = 2, 2048, 1024
H, DK, DV = 16, 64, 64
SCALE = float(np.sqrt(DK))
NCORES = 8
GROUPS = 4           # head-groups (cores per batch)
HPG = H // GROUPS    # heads per core = 4
CH = HPG * DK        # channels per core = 256
MC = CH // 128       # c-chunks = 2
DJ = D // 128        # contraction chunks = 8
NQC = S // 128       # 16
P = 128

_BUILD_CACHE = {}
LAST_RESULTS = None  # test harness can read exec_time_ns etc. from here


def _round_fp32r(a: np.ndarray) -> np.ndarray:
    """Round fp32 to fp32r (11-bit mantissa, RTNE); layout stays fp32."""
    u = np.ascontiguousarray(a, dtype=np.float32).view(np.uint32)
    low = u & np.uint32(0xFFF)
    hi = u & np.uint32(0xFFFFF000)
    round_up = (low > 0x800) | ((low == 0x800) & (((hi >> np.uint32(12)) & np.uint32(1)) == 1))
    hi = hi + (round_up.astype(np.uint32) << np.uint32(12))
    return hi.view(np.float32)


def _build(n_kp: int):
    """Build + schedule the per-core Bass program for a padded key count."""
    import concourse.bass as bass  # noqa: F401
    from concourse import bacc, tile, mybir

    DT = mybir.dt
    F32, F32R, BF16 = DT.float32, DT.float32r, DT.bfloat16
    AF = mybir.ActivationFunctionType
    ALU = mybir.AluOpType

    NJ = n_kp // P                      # k-chunks
    NKB = (n_kp + 511) // 512           # 512-wide k blocks for the k projection

    nc = bacc.Bacc("TRN2", target_bir_lowering=False, debug=False,
                   num_devices=NCORES)

    xqT = nc.dram_tensor("xqT", [D, S], F32R, kind="ExternalInput")
    xkT = nc.dram_tensor("xkT", [D, n_kp], F32R, kind="ExternalInput")
    xvT = nc.dram_tensor("xvT", [D, n_kp], F32R, kind="ExternalInput")
    wqT = nc.dram_tensor("wqT", [D, CH], F32R, kind="ExternalInput")
    wkT = nc.dram_tensor("wkT", [D, CH], F32R, kind="ExternalInput")
    wvT = nc.dram_tensor("wvT", [D, CH], F32R, kind="ExternalInput")
    woT = nc.dram_tensor("woT", [CH, D], F32R, kind="ExternalInput")
    bq = nc.dram_tensor("bq", [CH], F32, kind="ExternalInput")
    bk = nc.dram_tensor("bk", [CH], F32, kind="ExternalInput")
    bv = nc.dram_tensor("bv", [CH], F32, kind="ExternalInput")
    valid = nc.dram_tensor("valid", [n_kp], F32, kind="ExternalInput")
    out = nc.dram_tensor("out", [S, D], F32, kind="ExternalOutput")

    with tile.TileContext(nc) as tc:
        with (
            tc.tile_pool(name="xs", bufs=10) as xs,
            tc.tile_pool(name="persist", bufs=1) as pp,
            tc.tile_pool(name="exps", bufs=4) as ep,
            tc.tile_pool(name="scratch", bufs=3) as scr,
            tc.tile_pool(name="outs", bufs=3) as op,
            tc.tile_pool(name="smalls", bufs=4) as smalls,
            tc.tile_pool(name="cu", bufs=3) as cu,
            tc.tile_pool(name="psw", bufs=2, space="PSUM") as psw,
            tc.tile_pool(name="psc", bufs=2, space="PSUM") as psc,
            tc.tile_pool(name="dscr", bufs=2, space="DRAM") as dscr,
        ):
            # ---- constants / weights (DMA issue order == consumption order:
            # wk+xk feed the first matmuls, so they go first) ---------------
            wq_sb = pp.tile([P, DJ, CH], F32R, name="wq_sb")
            wk_sb = pp.tile([P, DJ, CH], F32R, name="wk_sb")
            wv_sb = pp.tile([P, DJ, CH], F32R, name="wv_sb")
            wo_sb = pp.tile([P, MC, D], F32R, name="wo_sb")
            bq_sb = pp.tile([P, MC], F32, name="bq_sb")
            bk_sb = pp.tile([P, MC], F32, name="bk_sb")
            qT_sb = pp.tile([P, MC, S], F32R, name="qT_sb")
            kT_sb = pp.tile([P, MC, n_kp], F32R, name="kT_sb")
            vaug = pp.tile([P, NJ, HPG, DV + 1], BF16, name="vaug")
            ctxN = pp.tile([P, MC, S], F32R, name="ctxN")

            # weight loads split per 128-row chunk (contiguous DRAM reads beat
            # one big strided DMA); chunk 0 is issued before the X stream so
            # the first projection matmul can start as soon as xk[0] lands.
            nc.sync.dma_start(out=wk_sb[:, 0, :], in_=wkT.ap()[0:P, :])
            nc.sync.dma_start(out=bk_sb[:], in_=bk.ap().rearrange("(m p) -> p m", p=P))

            # ---- k projection: kT[c, s] = sum_d WkT[d,c] * XkT[d,s] (+bk) --
            xk_t = [xs.tile([P, S], F32R, tag="x", name=f"xk{dj}") for dj in range(DJ)]
            nc.sync.dma_start(out=xk_t[0][:, :n_kp], in_=xkT.ap()[0:P, :])
            for dj in range(1, DJ):
                nc.sync.dma_start(out=wk_sb[:, dj, :], in_=wkT.ap()[dj * P:(dj + 1) * P, :])
                nc.sync.dma_start(out=xk_t[dj][:, :n_kp], in_=xkT.ap()[dj * P:(dj + 1) * P, :])
            nc.sync.dma_start(out=bq_sb[:], in_=bq.ap().rearrange("(m p) -> p m", p=P))
            bv_rep = pp.tile([P, CH], F32, name="bv_rep")
            nc.gpsimd.dma_start(out=bv_rep[:], in_=bv.ap()[None, :].partition_broadcast(P))
            valid_sb = pp.tile([P, NJ], F32, name="valid_sb")
            nc.sync.dma_start(out=valid_sb[:], in_=valid.ap().rearrange("(j p) -> p j", p=P))
            valid_bf = pp.tile([P, NJ], BF16, name="valid_bf")
            nc.vector.tensor_copy(out=valid_bf[:], in_=valid_sb[:])
            for m in range(MC):
                for kb in range(NKB):
                    w = min(512, n_kp - kb * 512)
                    ps = psw.tile([P, 1024], mybir.dt.float32, tag="ps")
                    for dj in range(DJ):
                        nc.tensor.matmul(
                            ps[:, :w],
                            lhsT=wk_sb[:, dj, m * P:(m + 1) * P],
                            rhs=xk_t[dj][:, kb * 512:kb * 512 + w],
                            start=(dj == 0), stop=(dj == DJ - 1))
                    nc.vector.tensor_scalar(
                        out=kT_sb[:, m, kb * 512:kb * 512 + w], in0=ps[:, :w],
                        scalar1=bk_sb[:, m:m + 1], scalar2=None, op0=ALU.add)

            # ---- q projection: qT[c, s] (scale already folded into Wq/bq) --
            xq_t = [xs.tile([P, S], F32R, tag="x", name=f"xq{dj}") for dj in range(DJ)]
            for dj in range(DJ):
                nc.sync.dma_start(out=wq_sb[:, dj, :], in_=wqT.ap()[dj * P:(dj + 1) * P, :])
                nc.sync.dma_start(out=xq_t[dj][:], in_=xqT.ap()[dj * P:(dj + 1) * P, :])
            for m in range(MC):
                for qb in range(S // 512):
                    ps = psw.tile([P, 1024], mybir.dt.float32, tag="ps")
                    for dj in range(DJ):
                        nc.tensor.matmul(
                            ps[:, :512],
                            lhsT=wq_sb[:, dj, m * P:(m + 1) * P],
                            rhs=xq_t[dj][:, qb * 512:(qb + 1) * 512],
                            start=(dj == 0), stop=(dj == DJ - 1))
                    nc.vector.tensor_scalar(
                        out=qT_sb[:, m, qb * 512:(qb + 1) * 512], in0=ps[:, :512],
                        scalar1=bq_sb[:, m:m + 1], scalar2=None, op0=ALU.add)

            # ---- v projection: v[s, c] (+bv, *valid), build V_aug ---------
            xv_t = [xs.tile([P, S], F32R, tag="x", name=f"xv{dj}") for dj in range(DJ)]
            for dj in range(DJ):
                nc.sync.dma_start(out=wv_sb[:, dj, :], in_=wvT.ap()[dj * P:(dj + 1) * P, :])
                nc.sync.dma_start(out=xv_t[dj][:, :n_kp], in_=xvT.ap()[dj * P:(dj + 1) * P, :])
            for m2 in range(MC):
                nc.sync.dma_start(out=wo_sb[:, m2, :], in_=woT.ap()[m2 * P:(m2 + 1) * P, :])
            for j in range(NJ):
                ps = psw.tile([P, 1024], mybir.dt.float32, tag="ps")
                for dj in range(DJ):
                    nc.tensor.matmul(
                        ps[:, :CH],
                        lhsT=xv_t[dj][:, j * P:(j + 1) * P],
                        rhs=wv_sb[:, dj, :],
                        start=(dj == 0), stop=(dj == DJ - 1))
                vst = scr.tile([P, 1024], mybir.dt.float32, tag="s")
                nc.vector.tensor_tensor(out=vst[:, :CH], in0=ps[:, :CH], in1=bv_rep[:], op=ALU.add)
                nc.vector.tensor_scalar(
                    out=vaug[:, j, :, 0:DV],
                    in0=vst[:, :CH].rearrange("p (h d) -> p h d", h=HPG),
                    scalar1=valid_sb[:, j:j + 1], scalar2=None, op0=ALU.mult)
                for h in range(HPG):
                    nc.gpsimd.tensor_copy(out=vaug[:, j, h, DV:DV + 1], in_=valid_bf[:, j:j + 1])

            # ---- attention, processed per (q-half, head) -----------------
            # ST orientation: scores^T [k, q]; exp on ACT (PSUM -> bf16 SBUF);
            # AV accumulates ctx^T (+denominator row 64) per 1024-wide q half.
            def emit_outproj(qc, evac_engine="vector"):
                ps = psw.tile([P, 1024], mybir.dt.float32, tag="ps", name=f"ops{qc}")
                for n2 in range(2):
                    for m in range(MC):
                        nc.tensor.matmul(
                            ps[:, n2 * 512:(n2 + 1) * 512],
                            lhsT=ctxN[:, m, qc * P:(qc + 1) * P],
                            rhs=wo_sb[:, m, n2 * 512:(n2 + 1) * 512],
                            start=(m == 0), stop=(m == MC - 1))
                stage = op.tile([P, D], mybir.dt.float32, tag="o", name=f"og{qc}")
                if evac_engine == "scalar":
                    nc.scalar.copy(out=stage[:], in_=ps[:])
                else:
                    nc.vector.tensor_copy(out=stage[:], in_=ps[:])
                nc.sync.dma_start(out=out.ap()[qc * P:(qc + 1) * P, :], in_=stage[:])

            def emit_attention(half, h):
                q0 = half * 1024
                m, po = h // 2, (h % 2) * 64
                ctx_ps = psc.tile([P, 1024], mybir.dt.float32, tag="ctx",
                                  name=f"ctx{half}{h}")
                # one-step software skew: emit AV(j-1) after ST(j)/exp(j) so
                # the in-order PE stream always has independent score matmuls
                # queued while the exp it needs is still running on ScalarE.
                def emit_av(j, ex):
                    for qq in range(2):
                        nc.tensor.matmul(
                            ctx_ps[0:DV + 1, qq * 512:(qq + 1) * 512],
                            lhsT=vaug[:, j, h, :],
                            rhs=ex[:, qq * 512:(qq + 1) * 512],
                            start=(j == 0), stop=(j == NJ - 1))

                pending = None
                for j in range(NJ):
                    st = psw.tile([P, 1024], mybir.dt.float32, tag="ps",
                                  name=f"st{half}{h}{j}")
                    for qq in range(2):
                        nc.tensor.matmul(
                            st[:, qq * 512:(qq + 1) * 512],
                            lhsT=kT_sb[po:po + 64, m, j * P:(j + 1) * P],
                            rhs=qT_sb[po:po + 64, m, q0 + qq * 512:q0 + (qq + 1) * 512],
                            start=True, stop=True)
                    ex = ep.tile([P, 1024], BF16, tag="e", name=f"ex{half}{h}{j}")
                    nc.scalar.activation(out=ex[:], in_=st[:], func=AF.Exp)
                    if pending is not None:
                        emit_av(*pending)
                    pending = (j, ex)
                emit_av(*pending)
                # Evacuate unnormalized ctx^T (and the denominator row) to
                # SBUF right away so the PSUM slot frees fast; the reciprocal
                # chain below then runs entirely off the PE's critical path.
                ctxU = cu.tile([P, 1024], mybir.dt.float32, tag="cu",
                               name=f"cu{half}{h}")
                nc.vector.tensor_copy(out=ctxU[0:DV + 1, :], in_=ctx_ps[0:DV + 1, :])
                # reciprocal of the denominator on a [128, 8] reshape (a
                # [1, 1024] DVE reciprocal is ~13us); DRAM bounces reshape.
                rb = dscr.tile([1, 1024], mybir.dt.float32, tag="rb")
                nc.sync.dma_start(out=rb[:], in_=ctxU[64:65, :])
                rsq = smalls.tile([P, 8], mybir.dt.float32, tag="rsq")
                nc.sync.dma_start(out=rsq[:], in_=rb.rearrange("o (p a) -> (o p) a", p=P))
                rcq = smalls.tile([P, 8], mybir.dt.float32, tag="rcq")
                nc.vector.reciprocal(out=rcq[:], in_=rsq[:])
                rb2 = dscr.tile([1, 1024], mybir.dt.float32, tag="rb2")
                nc.sync.dma_start(out=rb2.rearrange("o (p a) -> (o p) a", p=P), in_=rcq[:])
                rec = scr.tile([P, 1024], mybir.dt.float32, tag="s", name=f"rc{half}{h}")
                nc.gpsimd.dma_start(out=rec[0:64, :],
                                    in_=rb2[0][None, :].partition_broadcast(64))
                if po == 0:
                    nc.vector.tensor_tensor(out=ctxN[0:64, m, q0:q0 + 1024],
                                            in0=ctxU[0:64, :],
                                            in1=rec[0:64, :], op=ALU.mult)
                else:
                    tmp = scr.tile([P, 1024], F32R, tag="s", name=f"tm{half}{h}")
                    nc.vector.tensor_tensor(out=tmp[0:64, :],
                                            in0=ctxU[0:64, :],
                                            in1=rec[0:64, :], op=ALU.mult)
                    nc.sync.dma_start(out=ctxN[64:128, m, q0:q0 + 1024],
                                      in_=tmp[0:64, :])

            # half 0 attention; its out-proj is interleaved into half 1's
            # emission so the PE stream always has independent work queued.
            for h in range(HPG):
                emit_attention(0, h)
            # half 1: odd heads first so the LAST normalize chain (which gates
            # the tail out-proj chunks) belongs to an even head and skips the
            # partition-shift DMA hop.
            for i, h in enumerate((1, 3, 0, 2)):
                emit_attention(1, h)
                for qc in range(i * 2, i * 2 + 2):
                    emit_outproj(qc)
            for qc in range(8, NQC):
                emit_outproj(qc, evac_engine="scalar")

    nc.compile()
    return nc


def _ensure_axon_hooks():
    """bass_utils imports antenv.axon_hooks when tracing; this image's antenv
    lacks it. Provide it, backed by the ctypes NTFF hook when available."""
    import sys
    import types
    try:
        import antenv.axon_hooks  # noqa: F401
        return
    except ImportError:
        pass
    hook = None
    try:
        from trn_agent_boot.trn_boot import _ntff_profile_via_ctypes
        hook = _ntff_profile_via_ctypes("/opt/axon/libaxon_pjrt.so")
    except Exception:
        hook = None
    mod = types.ModuleType("antenv.axon_hooks")
    mod._hook = hook
    mod.get_axon_ntff_profile_hook = lambda: mod._hook
    mod.set_axon_ntff_profile_hook = lambda h: setattr(mod, "_hook", h)
    sys.modules["antenv.axon_hooks"] = mod


def kernel(Q, K, V, atte_mask_out, Wq, bq, Wk, bk, Wv, bv, Wo, bo):
    import jax  # noqa: F401  (must be imported first so the axon backend registers)
    from concourse.bass_utils import run_bass_kernel_spmd
    global LAST_RESULTS
    _ensure_axon_hooks()

    Q = np.asarray(Q); K = np.asarray(K); V = np.asarray(V)
    mask = np.asarray(atte_mask_out).reshape(B, S)
    Wq = np.asarray(Wq); Wk = np.asarray(Wk); Wv = np.asarray(Wv); Wo = np.asarray(Wo)
    bq = np.asarray(bq); bk = np.asarray(bk); bv = np.asarray(bv); bo = np.asarray(bo)

    keep = [np.flatnonzero(~mask[b]) for b in range(B)]
    n_kp = max(P, max(((len(ix) + P - 1) // P) * P for ix in keep))

    # per-batch packed (and fp32r-rounded) tensors
    xqT, xkT, xvT, validv = [], [], [], []
    for b in range(B):
        ix = keep[b]
        xqT.append(_round_fp32r(Q[b].T))
        kk = np.zeros((D, n_kp), np.float32)
        vv = np.zeros((D, n_kp), np.float32)
        kk[:, :len(ix)] = K[b][ix].T
        vv[:, :len(ix)] = V[b][ix].T
        xkT.append(_round_fp32r(kk))
        xvT.append(_round_fp32r(vv))
        va = np.zeros(n_kp, np.float32)
        va[:len(ix)] = 1.0
        validv.append(va)

    in_maps = []
    for c in range(NCORES):
        b, g = c // GROUPS, c % GROUPS
        sl = slice(g * CH, (g + 1) * CH)
        in_maps.append({
            "xqT": xqT[b], "xkT": xkT[b], "xvT": xvT[b],
            "wqT": _round_fp32r(Wq[sl].T / SCALE),
            "wkT": _round_fp32r(Wk[sl].T),
            "wvT": _round_fp32r(Wv[sl].T),
            "woT": _round_fp32r(Wo[:, sl].T),
            "bq": np.ascontiguousarray(bq[sl] / SCALE, np.float32),
            "bk": np.ascontiguousarray(bk[sl], np.float32),
            "bv": np.ascontiguousarray(bv[sl], np.float32),
            "valid": validv[b],
        })

    if n_kp not in _BUILD_CACHE:
        _BUILD_CACHE[n_kp] = _build(n_kp)
    nc = _BUILD_CACHE[n_kp]

    res = run_bass_kernel_spmd(nc, in_maps, core_ids=list(range(NCORES)))
    LAST_RESULTS = res

    full = np.zeros((B, S, D), np.float32)
    full += bo.astype(np.float32)
    for c in range(NCORES):
        full[c // GROUPS] += res.results[c]["out"]
    return full



# revision 8
# speedup vs baseline: 1.3719x; 1.3719x over previous
"""Multi-head attention (B=2, S=2048, D=1024, H=16, dk=dv=64) on 8 TRN2 NeuronCores.

Sharding: core c -> (batch b = c//4, head-group g = c%4, 4 heads each).
Each core computes q/k/v projections for its 4 heads (weight-column shard),
attention over its batch, and a partial output projection over its 256
channels (weight-row shard of Wo).  The host sums the 4 partial outputs per
batch at unshard time (the "all-reduce after the output projection").

Host-side shard prep:
  * X slices are transposed to [D, S] so the contraction dim (D) lands on
    SBUF partitions for the projection matmuls.
  * The key-padding mask is applied by COMPACTION: masked keys are removed
    (gathered) from K/V before they ever reach the device.  This is exactly
    the reference semantics (masked keys get zero attention weight) and
    roughly halves the k-side work.
  * The softmax 1/sqrt(dk) scale is folded into Wq/bq.
  * All matmul operands are bf16: the PE streams bf16 at the same cycles/col
    as fp32r but at far lower power (all 8 cores share the chip power budget,
    and the NTFF throttle counters show fp32 matmuls pin the PE at a low
    DVFS state), and bf16 halves both HBM traffic and LDWEIGHTS time.

Device softmax: scores are small (|s| ~ 10) so exp needs no max-subtraction.
The denominator comes for free as a 65th "ones" column appended to V; the
normalization divides the unnormalized context rows by that accumulated sum.
The output bias bo is added on the host during the partial-sum gather.
"""
import ml_dtypes
import numpy as np

B, S, D = 2, 2048, 1024
H, DK, DV = 16, 64, 64
SCALE = float(np.sqrt(DK))
NCORES = 8
GROUPS = 4           # head-groups (cores per batch)
HPG = H // GROUPS    # heads per core = 4
CH = HPG * DK        # channels per core = 256
MC = CH // 128       # c-chunks = 2
DJ = D // 128        # contraction chunks = 8
NQC = S // 128       # 16
P = 128

_BUILD_CACHE = {}
LAST_RESULTS = None  # test harness can read exec_time_ns etc. from here


def _bf16(a: np.ndarray) -> np.ndarray:
    return np.ascontiguousarray(a, dtype=np.float32).astype(ml_dtypes.bfloat16)


def _build(n_kp: int):
    """Build + schedule the per-core Bass program for a padded key count."""
    import concourse.bass as bass  # noqa: F401
    from concourse import bacc, tile, mybir

    DT = mybir.dt
    F32, F32R, BF16 = DT.float32, DT.float32r, DT.bfloat16
    AF = mybir.ActivationFunctionType
    ALU = mybir.AluOpType

    NJ = n_kp // P                      # k-chunks
    NKB = (n_kp + 511) // 512           # 512-wide k blocks for the k projection

    nc = bacc.Bacc("TRN2", target_bir_lowering=False, debug=False,
                   num_devices=NCORES)

    xqT = nc.dram_tensor("xqT", [D, S], BF16, kind="ExternalInput")
    xkT = nc.dram_tensor("xkT", [D, n_kp], BF16, kind="ExternalInput")
    xvT = nc.dram_tensor("xvT", [D, n_kp], BF16, kind="ExternalInput")
    wqT = nc.dram_tensor("wqT", [D, CH], BF16, kind="ExternalInput")
    wkT = nc.dram_tensor("wkT", [D, CH], BF16, kind="ExternalInput")
    wvT = nc.dram_tensor("wvT", [D, CH], BF16, kind="ExternalInput")
    woT = nc.dram_tensor("woT", [CH, D], BF16, kind="ExternalInput")
    bq = nc.dram_tensor("bq", [CH], F32, kind="ExternalInput")
    bk = nc.dram_tensor("bk", [CH], F32, kind="ExternalInput")
    bv = nc.dram_tensor("bv", [CH], F32, kind="ExternalInput")
    valid = nc.dram_tensor("valid", [n_kp], F32, kind="ExternalInput")
    out = nc.dram_tensor("out", [S, D], F32, kind="ExternalOutput")

    with tile.TileContext(nc) as tc:
        with (
            tc.tile_pool(name="xs", bufs=10) as xs,
            tc.tile_pool(name="persist", bufs=1) as pp,
            tc.tile_pool(name="exps", bufs=4) as ep,
            tc.tile_pool(name="scratch", bufs=3) as scr,
            tc.tile_pool(name="outs", bufs=3) as op,
            tc.tile_pool(name="smalls", bufs=4) as smalls,
            tc.tile_pool(name="cu", bufs=3) as cu,
            tc.tile_pool(name="psw", bufs=2, space="PSUM") as psw,
            tc.tile_pool(name="psc", bufs=2, space="PSUM") as psc,
            tc.tile_pool(name="dscr", bufs=2, space="DRAM") as dscr,
        ):
            # ---- constants / weights (DMA issue order == consumption order:
            # wk+xk feed the first matmuls, so they go first) ---------------
            wq_sb = pp.tile([P, DJ, CH], BF16, name="wq_sb")
            wk_sb = pp.tile([P, DJ, CH], BF16, name="wk_sb")
            wv_sb = pp.tile([P, DJ, CH], BF16, name="wv_sb")
            wo_sb = pp.tile([P, MC, D], BF16, name="wo_sb")
            bq_sb = pp.tile([P, MC], F32, name="bq_sb")
            bk_sb = pp.tile([P, MC], F32, name="bk_sb")
            qT_sb = pp.tile([P, MC, S], BF16, name="qT_sb")
            kT_sb = pp.tile([P, MC, n_kp], BF16, name="kT_sb")
            vaug = pp.tile([P, NJ, HPG, DV + 1], BF16, name="vaug")
            ctxN = pp.tile([P, MC, S], BF16, name="ctxN")

            # weight loads split per 128-row chunk (contiguous DRAM reads beat
            # one big strided DMA); chunk 0 is issued before the X stream so
            # the first projection matmul can start as soon as xk[0] lands.
            nc.sync.dma_start(out=wk_sb[:, 0, :], in_=wkT.ap()[0:P, :])
            nc.sync.dma_start(out=bk_sb[:], in_=bk.ap().rearrange("(m p) -> p m", p=P))

            # ---- k projection: kT[c, s] = sum_d WkT[d,c] * XkT[d,s] (+bk) --
            xk_t = [xs.tile([P, S], BF16, tag="x", name=f"xk{dj}") for dj in range(DJ)]
            nc.sync.dma_start(out=xk_t[0][:, :n_kp], in_=xkT.ap()[0:P, :])
            for dj in range(1, DJ):
                nc.sync.dma_start(out=wk_sb[:, dj, :], in_=wkT.ap()[dj * P:(dj + 1) * P, :])
                nc.sync.dma_start(out=xk_t[dj][:, :n_kp], in_=xkT.ap()[dj * P:(dj + 1) * P, :])
            nc.sync.dma_start(out=bq_sb[:], in_=bq.ap().rearrange("(m p) -> p m", p=P))
            bv_rep = pp.tile([P, CH], F32, name="bv_rep")
            nc.gpsimd.dma_start(out=bv_rep[:], in_=bv.ap()[None, :].partition_broadcast(P))
            valid_sb = pp.tile([P, NJ], F32, name="valid_sb")
            nc.sync.dma_start(out=valid_sb[:], in_=valid.ap().rearrange("(j p) -> p j", p=P))
            valid_bf = pp.tile([P, NJ], BF16, name="valid_bf")
            nc.vector.tensor_copy(out=valid_bf[:], in_=valid_sb[:])
            for m in range(MC):
                for kb in range(NKB):
                    w = min(512, n_kp - kb * 512)
                    ps = psw.tile([P, 1024], mybir.dt.float32, tag="ps")
                    for dj in range(DJ):
                        nc.tensor.matmul(
                            ps[:, :w],
                            lhsT=wk_sb[:, dj, m * P:(m + 1) * P],
                            rhs=xk_t[dj][:, kb * 512:kb * 512 + w],
                            start=(dj == 0), stop=(dj == DJ - 1))
                    nc.vector.tensor_scalar(
                        out=kT_sb[:, m, kb * 512:kb * 512 + w], in0=ps[:, :w],
                        scalar1=bk_sb[:, m:m + 1], scalar2=None, op0=ALU.add)

            # ---- q projection: qT[c, s] (scale already folded into Wq/bq) --
            xq_t = [xs.tile([P, S], BF16, tag="x", name=f"xq{dj}") for dj in range(DJ)]
            for dj in range(DJ):
                nc.sync.dma_start(out=wq_sb[:, dj, :], in_=wqT.ap()[dj * P:(dj + 1) * P, :])
                nc.sync.dma_start(out=xq_t[dj][:], in_=xqT.ap()[dj * P:(dj + 1) * P, :])
            for m in range(MC):
                for qb in range(S // 512):
                    ps = psw.tile([P, 1024], mybir.dt.float32, tag="ps")
                    for dj in range(DJ):
                        nc.tensor.matmul(
                            ps[:, :512],
                            lhsT=wq_sb[:, dj, m * P:(m + 1) * P],
                            rhs=xq_t[dj][:, qb * 512:(qb + 1) * 512],
                            start=(dj == 0), stop=(dj == DJ - 1))
                    nc.vector.tensor_scalar(
                        out=qT_sb[:, m, qb * 512:(qb + 1) * 512], in0=ps[:, :512],
                        scalar1=bq_sb[:, m:m + 1], scalar2=None, op0=ALU.add)

            # ---- v projection: v[s, c] (+bv, *valid), build V_aug ---------
            xv_t = [xs.tile([P, S], BF16, tag="x", name=f"xv{dj}") for dj in range(DJ)]
            for dj in range(DJ):
                nc.sync.dma_start(out=wv_sb[:, dj, :], in_=wvT.ap()[dj * P:(dj + 1) * P, :])
                nc.sync.dma_start(out=xv_t[dj][:, :n_kp], in_=xvT.ap()[dj * P:(dj + 1) * P, :])
            for m2 in range(MC):
                nc.sync.dma_start(out=wo_sb[:, m2, :], in_=woT.ap()[m2 * P:(m2 + 1) * P, :])
            for j in range(NJ):
                ps = psw.tile([P, 1024], mybir.dt.float32, tag="ps")
                for dj in range(DJ):
                    nc.tensor.matmul(
                        ps[:, :CH],
                        lhsT=xv_t[dj][:, j * P:(j + 1) * P],
                        rhs=wv_sb[:, dj, :],
                        start=(dj == 0), stop=(dj == DJ - 1))
                vst = scr.tile([P, 1024], mybir.dt.float32, tag="s")
                nc.vector.tensor_tensor(out=vst[:, :CH], in0=ps[:, :CH], in1=bv_rep[:], op=ALU.add)
                nc.vector.tensor_scalar(
                    out=vaug[:, j, :, 0:DV],
                    in0=vst[:, :CH].rearrange("p (h d) -> p h d", h=HPG),
                    scalar1=valid_sb[:, j:j + 1], scalar2=None, op0=ALU.mult)
                for h in range(HPG):
                    nc.gpsimd.tensor_copy(out=vaug[:, j, h, DV:DV + 1], in_=valid_bf[:, j:j + 1])

            # ---- attention, processed per (q-half, head) -----------------
            # ST orientation: scores^T [k, q]; exp on ACT (PSUM -> bf16 SBUF);
            # AV accumulates ctx^T (+denominator row 64) per 1024-wide q half.
            def emit_outproj(qc, evac_engine="vector"):
                ps = psw.tile([P, 1024], mybir.dt.float32, tag="ps", name=f"ops{qc}")
                for n2 in range(2):
                    for m in range(MC):
                        nc.tensor.matmul(
                            ps[:, n2 * 512:(n2 + 1) * 512],
                            lhsT=ctxN[:, m, qc * P:(qc + 1) * P],
                            rhs=wo_sb[:, m, n2 * 512:(n2 + 1) * 512],
                            start=(m == 0), stop=(m == MC - 1))
                stage = op.tile([P, D], mybir.dt.float32, tag="o", name=f"og{qc}")
                if evac_engine == "scalar":
                    nc.scalar.copy(out=stage[:], in_=ps[:])
                else:
                    nc.vector.tensor_copy(out=stage[:], in_=ps[:])
                nc.sync.dma_start(out=out.ap()[qc * P:(qc + 1) * P, :], in_=stage[:])

            def emit_attention(half, h):
                q0 = half * 1024
                m, po = h // 2, (h % 2) * 64
                ctx_ps = psc.tile([P, 1024], mybir.dt.float32, tag="ctx",
                                  name=f"ctx{half}{h}")
                # one-step software skew: emit AV(j-1) after ST(j)/exp(j) so
                # the in-order PE stream always has independent score matmuls
                # queued while the exp it needs is still running on ScalarE.
                def emit_av(j, ex):
                    for qq in range(2):
                        nc.tensor.matmul(
                            ctx_ps[0:DV + 1, qq * 512:(qq + 1) * 512],
                            lhsT=vaug[:, j, h, :],
                            rhs=ex[:, qq * 512:(qq + 1) * 512],
                            start=(j == 0), stop=(j == NJ - 1))

                pending = None
                for j in range(NJ):
                    st = psw.tile([P, 1024], mybir.dt.float32, tag="ps",
                                  name=f"st{half}{h}{j}")
                    for qq in range(2):
                        nc.tensor.matmul(
                            st[:, qq * 512:(qq + 1) * 512],
                            lhsT=kT_sb[po:po + 64, m, j * P:(j + 1) * P],
                            rhs=qT_sb[po:po + 64, m, q0 + qq * 512:q0 + (qq + 1) * 512],
                            start=True, stop=True)
                    ex = ep.tile([P, 1024], BF16, tag="e", name=f"ex{half}{h}{j}")
                    nc.scalar.activation(out=ex[:], in_=st[:], func=AF.Exp)
                    if pending is not None:
                        emit_av(*pending)
                    pending = (j, ex)
                emit_av(*pending)
                # Evacuate unnormalized ctx^T (and the denominator row) to
                # SBUF right away so the PSUM slot frees fast; the reciprocal
                # chain below then runs entirely off the PE's critical path.
                ctxU = cu.tile([P, 1024], mybir.dt.float32, tag="cu",
                               name=f"cu{half}{h}")
                nc.vector.tensor_copy(out=ctxU[0:DV + 1, :], in_=ctx_ps[0:DV + 1, :])
                # reciprocal of the denominator on a [128, 8] reshape (a
                # [1, 1024] DVE reciprocal is ~13us); DRAM bounces reshape.
                rb = dscr.tile([1, 1024], mybir.dt.float32, tag="rb")
                nc.sync.dma_start(out=rb[:], in_=ctxU[64:65, :])
                rsq = smalls.tile([P, 8], mybir.dt.float32, tag="rsq")
                nc.sync.dma_start(out=rsq[:], in_=rb.rearrange("o (p a) -> (o p) a", p=P))
                rcq = smalls.tile([P, 8], mybir.dt.float32, tag="rcq")
                nc.vector.reciprocal(out=rcq[:], in_=rsq[:])
                rb2 = dscr.tile([1, 1024], mybir.dt.float32, tag="rb2")
                nc.sync.dma_start(out=rb2.rearrange("o (p a) -> (o p) a", p=P), in_=rcq[:])
                rec = scr.tile([P, 1024], mybir.dt.float32, tag="s", name=f"rc{half}{h}")
                nc.gpsimd.dma_start(out=rec[0:64, :],
                                    in_=rb2[0][None, :].partition_broadcast(64))
                if po == 0:
                    nc.vector.tensor_tensor(out=ctxN[0:64, m, q0:q0 + 1024],
                                            in0=ctxU[0:64, :],
                                            in1=rec[0:64, :], op=ALU.mult)
                else:
                    tmp = scr.tile([P, 1024], BF16, tag="s", name=f"tm{half}{h}")
                    nc.vector.tensor_tensor(out=tmp[0:64, :],
                                            in0=ctxU[0:64, :],
                                            in1=rec[0:64, :], op=ALU.mult)
                    nc.sync.dma_start(out=ctxN[64:128, m, q0:q0 + 1024],
                                      in_=tmp[0:64, :])

            # half 0 attention; its out-proj is interleaved into half 1's
            # emission so the PE stream always has independent work queued.
            for h in range(HPG):
                emit_attention(0, h)
            # half 1: odd heads first so the LAST normalize chain (which gates
            # the tail out-proj chunks) belongs to an even head and skips the
            # partition-shift DMA hop.
            for i, h in enumerate((1, 3, 0, 2)):
                emit_attention(1, h)
                for qc in range(i * 2, i * 2 + 2):
                    emit_outproj(qc)
            for qc in range(8, NQC):
                emit_outproj(qc, evac_engine="scalar")

    nc.compile()
    return nc


def _ensure_axon_hooks():
    """bass_utils imports antenv.axon_hooks when tracing; this image's antenv
    lacks it. Provide it, backed by the ctypes NTFF hook when available."""
    import sys
    import types
    try:
        import antenv.axon_hooks  # noqa: F401
        return
    except ImportError:
        pass
    hook = None
    try:
        from trn_agent_boot.trn_boot import _ntff_profile_via_ctypes
        hook = _ntff_profile_via_ctypes("/opt/axon/libaxon_pjrt.so")
    except Exception:
        hook = None
    mod = types.ModuleType("antenv.axon_hooks")
    mod._hook = hook
    mod.get_axon_ntff_profile_hook = lambda: mod._hook
    mod.set_axon_ntff_profile_hook = lambda h: setattr(mod, "_hook", h)
    sys.modules["antenv.axon_hooks"] = mod


def kernel(Q, K, V, atte_mask_out, Wq, bq, Wk, bk, Wv, bv, Wo, bo):
    import jax  # noqa: F401  (must be imported first so the axon backend registers)
    from concourse.bass_utils import run_bass_kernel_spmd
    global LAST_RESULTS
    _ensure_axon_hooks()

    Q = np.asarray(Q); K = np.asarray(K); V = np.asarray(V)
    mask = np.asarray(atte_mask_out).reshape(B, S)
    Wq = np.asarray(Wq); Wk = np.asarray(Wk); Wv = np.asarray(Wv); Wo = np.asarray(Wo)
    bq = np.asarray(bq); bk = np.asarray(bk); bv = np.asarray(bv); bo = np.asarray(bo)

    keep = [np.flatnonzero(~mask[b]) for b in range(B)]
    n_kp = max(P, max(((len(ix) + P - 1) // P) * P for ix in keep))

    # per-batch packed (and bf16-rounded) tensors
    xqT, xkT, xvT, validv = [], [], [], []
    for b in range(B):
        ix = keep[b]
        xqT.append(_bf16(Q[b].T))
        kk = np.zeros((D, n_kp), np.float32)
        vv = np.zeros((D, n_kp), np.float32)
        kk[:, :len(ix)] = K[b][ix].T
        vv[:, :len(ix)] = V[b][ix].T
        xkT.append(_bf16(kk))
        xvT.append(_bf16(vv))
        va = np.zeros(n_kp, np.float32)
        va[:len(ix)] = 1.0
        validv.append(va)

    in_maps = []
    for c in range(NCORES):
        b, g = c // GROUPS, c % GROUPS
        sl = slice(g * CH, (g + 1) * CH)
        in_maps.append({
            "xqT": xqT[b], "xkT": xkT[b], "xvT": xvT[b],
            "wqT": _bf16(Wq[sl].T / SCALE),
            "wkT": _bf16(Wk[sl].T),
            "wvT": _bf16(Wv[sl].T),
            "woT": _bf16(Wo[:, sl].T),
            "bq": np.ascontiguousarray(bq[sl] / SCALE, np.float32),
            "bk": np.ascontiguousarray(bk[sl], np.float32),
            "bv": np.ascontiguousarray(bv[sl], np.float32),
            "valid": validv[b],
        })

    if n_kp not in _BUILD_CACHE:
        _BUILD_CACHE[n_kp] = _build(n_kp)
    nc = _BUILD_CACHE[n_kp]

    res = run_bass_kernel_spmd(nc, in_maps, core_ids=list(range(NCORES)))
    LAST_RESULTS = res

    full = np.zeros((B, S, D), np.float32)
    full += bo.astype(np.float32)
    for c in range(NCORES):
        full[c // GROUPS] += res.results[c]["out"]
    return full

